# revision 36
# baseline (speedup 1.0000x reference)
"""GhostAttention (B=2, T=2048, C=2048, H=16) on 8 Trainium2 NeuronCores.

Sharding: tensor-parallel over heads (Megatron-style). Core c owns heads
{2c, 2c+1}: it gets the 256 matching rows of Wq/Wk/Wv (column-parallel) and
the 256 matching columns of Wo (row-parallel), computes QKV projections,
masked-relu attention and its partial output projection for both batches,
and writes a full-shape partial y. The host sums the 8 partials.

v2 (bf16 + PE-continuity schedule):
  All matmul operands are bf16 (same PE rate as fp32r, half the SBUF/DMA
  traffic; enables fast DVE ops on 16-bit tiles). The attention scale is
  folded into Wq on the host.
  phase 1: the batch's full x^T lives in SBUF (64KB/partition in bf16), so
           each projection quantity (q/k per head, v per 128-token block)
           accumulates as its own full-bank PSUM group through a 2-bank
           ring -- PSUM allows only one accumulation group per 2KB bank.
           No drain bubbles; drains alternate ACT/DVE; weights arrive in
           4 k-groups so the first matmul starts ~2us in.
  phase 2: S^T blocks (tk=128, tq=512) with the S matmul emitted one block
           ahead of the relu+AV pair; relu (bias folded) alternates between
           ACT and DVE so drain throughput ~2x the PE block rate; diagonal
           blocks get a 0/1 mask multiply on DVE (bf16, 4x mode). AV and a
           ones-column normalizer matmul accumulate per block; the
           reciprocal is broadcast with a rank-1 matmul and applied on DVE
           directly PSUM*PSUM -> bf16 attn tile.
  phase 3: out-projection interleaved between the two head-groups of the
           next j-tile to keep the PE queue deep; PSUM drains alternate
           ACT/DVE; y staged in f32 and DMA'd per 128-token row block.
"""

import math
import sys

if "/opt/trn_rl_repo" not in sys.path:
    sys.path.insert(0, "/opt/trn_rl_repo")

import numpy as np
from contextlib import ExitStack

import concourse.bass as bass
import concourse.mybir as mybir
import concourse.tile as tile
from concourse.bass import ts, ds
from concourse.bass_utils import run_bass_kernel_spmd
from concourse.vector_clock import ScopedClock, VectorClock


def _split_drain_and_barrier(self, tick_clock, wait_clock):
    # This image's walrus caps sem waits per instruction; split the Tile-tail
    # drain waits across single-wait SP nops instead.
    gc = tick_clock.global_clock
    n = len(gc)
    for proc in range(n):
        t = gc[proc]
        if t <= 0:
            continue
        vc = VectorClock([0] * n)
        vc.require_at_least(proc, t)
        nop_inst = self.nc.sync.nop()
        wait_clock.add_sem_waits(nop_inst.ins, ScopedClock({None: vc}))
    self.nc.sync.drain()
    self.nc.all_engine_barrier()
    assert self.sems is not None
    popped = self.nc._tile_sem_poison_stack.pop()
    assert popped is self._sem_poison
    self.nc.clear_and_free_semaphores(list(self.sems.allocated().values()))
    self.nc.all_engine_barrier()


tile.TileContext._drain_and_barrier = _split_drain_and_barrier

_ws_counter = [0]


def split_excess_waits(nc, max_waits=1):
    """Hoist extra per-instruction sem waits onto preceding same-engine NoOps
    (same queue => they execute, and therefore wait, before the instruction)."""
    for fn in nc.m.functions:
        for blk in fn.blocks:
            insts = list(blk.instructions)
            out = []
            changed = False
            for inst in insts:
                si = inst.sync_info
                if si is not None and si.on_wait and len(si.on_wait) > max_waits:
                    waits = list(si.on_wait)
                    extra, keep = waits[:-max_waits], waits[-max_waits:]
                    for s in range(0, len(extra), max_waits):
                        chunk = extra[s : s + max_waits]
                        _ws_counter[0] += 1
                        nop = mybir.InstNoOp(
                            name=f"I-ws-{_ws_counter[0]}",
                            engine=inst.engine,
                            ins=[],
                            outs=[],
                            sync_info=mybir.SyncInfo(on_wait=chunk, on_update=[]),
                        )
                        out.append(nop)
                    inst.sync_info = mybir.SyncInfo(
                        on_wait=keep, on_update=list(si.on_update)
                    )
                    changed = True
                out.append(inst)
            if changed:
                try:
                    blk.instructions[:] = out
                except Exception:
                    blk.set_instructions(out)
    return nc


B, T, C = 2, 2048, 2048
H = 16
HD = C // H  # 128
N_CORES = 8
H_PER_CORE = H // N_CORES  # 2
CH = HD * H_PER_CORE  # 256 channels per core
SCALE = 1.0 / math.sqrt(HD)
ATTN_BIAS = 0.1  # relu(scores - (-0.1)) = relu(scores + 0.1)
EPS = 1e-6

F32 = mybir.dt.float32
F32R = mybir.dt.float32r
BF = mybir.dt.bfloat16
AF = mybir.ActivationFunctionType
ALU = mybir.AluOpType

_NC_CACHE = None

KT = C // 128  # 16 contraction slices
NCH = T // 256  # 8 phase-1 chunks per batch
NT = T // 512  # 4 query tiles of 512 per batch
USE_F32R_J0 = False  # f32r j=0 scores: numerically right in sim, wrong on HW
HILO_J0 = False  # bisect flag: hi+lo compensated (j0,i0) S block


def _build(split_waits=True):
    nc = bass.Bass("TRN2", debug=False)
    xT = nc.dram_tensor("xT", [C, B * T], BF, kind="ExternalInput")
    wq = nc.dram_tensor("wq", [C, CH], BF, kind="ExternalInput")  # pre-scaled
    wk = nc.dram_tensor("wk", [C, CH], BF, kind="ExternalInput")
    wv = nc.dram_tensor("wv", [C, CH], BF, kind="ExternalInput")
    wo = nc.dram_tensor("wo", [CH, C], BF, kind="ExternalInput")
    masks = nc.dram_tensor("masks", [4, 128, 512], BF, kind="ExternalInput")
    y = nc.dram_tensor("y", [B * T, C], F32, kind="ExternalOutput")

    with tile.TileContext(nc) as tc, ExitStack() as ctx:
        consts = ctx.enter_context(tc.tile_pool(name="consts", bufs=1))
        qkvp = ctx.enter_context(tc.tile_pool(name="qkv", bufs=1))
        xinp = ctx.enter_context(tc.tile_pool(name="xin", bufs=1))
        wp = ctx.enter_context(tc.tile_pool(name="wtile", bufs=4))
        attnp = ctx.enter_context(tc.tile_pool(name="attn", bufs=2))
        ystp = ctx.enter_context(tc.tile_pool(name="yst", bufs=2))
        smallp = ctx.enter_context(tc.tile_pool(name="small", bufs=2))
        recp = ctx.enter_context(tc.tile_pool(name="recp", bufs=4))
        posp = ctx.enter_context(tc.tile_pool(name="posb", bufs=4))
        resp = ctx.enter_context(tc.tile_pool(name="resid", bufs=2))

        wq_sb = consts.tile([128, KT, CH], BF, name="wq_sb", tag="wq")
        wk_sb = consts.tile([128, KT, CH], BF, name="wk_sb", tag="wk")
        wv_sb = consts.tile([128, KT, CH], BF, name="wv_sb", tag="wv")
        # 4 k-slice groups per weight so the first matmuls start early.
        for g in range(4):
            rs = ds(512 * g, 512)
            gs = ds(4 * g, 4)
            for w_d, w_s in ((wq_sb, wq), (wk_sb, wk), (wv_sb, wv)):
                nc.sync.dma_start(
                    w_d[:, gs, :],
                    w_s.ap()[rs, :].rearrange("(k p) o -> p k o", p=128),
                )
        wo_sb = consts.tile([128, H_PER_CORE, C], BF, name="wo_sb", tag="wo")
        nc.sync.dma_start(wo_sb[:], wo.ap().rearrange("(h p) o -> p h o", p=128))
        mask_sb = consts.tile([128, 4, 512], BF, name="mask_sb", tag="masks")
        for r in range(4):
            nc.sync.dma_start(mask_sb[:, r, :], masks.ap()[r])
        ones_col = consts.tile([128, 1], BF, name="ones_col", tag="ones_col")
        nc.vector.memset(ones_col[:], 1.0)
        ones_row_b = consts.tile([1, 128], BF, name="ones_row_b", tag="ones_row_b")
        nc.vector.memset(ones_row_b[:], 1.0)
        bias_sb = consts.tile([128, 1], F32, name="bias_sb", tag="bias")
        nc.vector.memset(bias_sb[:], ATTN_BIAS)

        xT_re = xT.ap().rearrange("(k p) t -> p k t", p=128)  # (128, KT, B*T)

        # global ACT/DVE alternation for PSUM drains
        par = [0]

        def drain(dst, src):
            # 3:2 ACT:DVE -- DVE also carries the reciprocal + mask work
            if par[0] % 5 < 3:
                nc.scalar.copy(dst, src)
            else:
                nc.vector.tensor_scalar_add(dst, src, 0.0)
            par[0] += 1

        relu_ctr = [0]

        def relu_drain(w_t, psb):
            # 3:2 ACT:DVE split -- ACT has more slack than DVE
            if relu_ctr[0] % 5 < 3:
                nc.scalar.activation(
                    w_t[:], psb[:], AF.Relu, bias=bias_sb[:], scale=1.0
                )
            else:
                nc.vector.tensor_scalar(
                    w_t[:], psb[:], ATTN_BIAS, 0.0, ALU.add, ALU.max
                )
            relu_ctr[0] += 1

        for b in range(B):
            q_sb = qkvp.tile([128, H_PER_CORE, T], BF, name="q_sb", tag="q")
            k_sb = qkvp.tile([128, H_PER_CORE, T], BF, name="k_sb", tag="k")
            v_sb = qkvp.tile([128, T // 128, CH], BF, name="v_sb", tag="v")
            # bf16 residuals of q/k for the first 512 tokens: rows t<128 have
            # a tiny normalizer, so the S block (j=0,i=0) runs hi+lo
            # compensated (3 bf16 matmuls) to get near-f32 scores there.
            # Own pool with bufs=2: no cross-batch buffer reuse.
            ql_sb = resp.tile([128, H_PER_CORE, 512], BF, name="ql", tag="ql")
            kl_sb = resp.tile([128, H_PER_CORE, 512], BF, name="kl", tag="kl")

            # ---- phase 1: x resident in SBUF; one PSUM group per bank ----
            xb = xinp.tile([128, KT, T], BF, name="xb", tag="xb")
            for n in range(NT):
                for kk in range(KT):
                    nc.sync.dma_start(
                        xb[:, kk, ts(n, 512)],
                        xT_re[:, kk, ds(T * b + 512 * n, 512)],
                    )
            with tc.tile_pool(name="ps1", bufs=2, space="PSUM") as pp1:
                for n in range(NT):
                    for w_sb, dst, dst32 in (
                        (wk_sb, k_sb, kl_sb),
                        (wq_sb, q_sb, ql_sb),
                    ):
                        for h in (0, 1):
                            pqk = pp1.tile([128, 512], F32, name="pqk", tag="pqk")
                            for kk in range(KT):
                                nc.tensor.matmul(
                                    pqk[:],
                                    w_sb[:, kk, ts(h, 128)],
                                    xb[:, kk, ds(512 * n, 512)],
                                    start=kk == 0,
                                    stop=kk == KT - 1,
                                )
                            if n == 0:
                                # single PSUM read to f32 SBUF, then derive
                                # bf16 hi + bf16 residual lo from SBUF
                                qf = resp.tile(
                                    [128, 512], F32, name="qf32", tag="qf32"
                                )
                                nc.scalar.copy(qf[:], pqk[:])
                                nc.scalar.copy(dst[:, h, ts(0, 512)], qf[:])
                                nc.vector.scalar_tensor_tensor(
                                    dst32[:, h, :],
                                    qf[:],
                                    1.0,
                                    dst[:, h, ts(0, 512)],
                                    op0=ALU.mult,
                                    op1=ALU.subtract,
                                )
                            else:
                                drain(dst[:, h, ts(n, 512)], pqk[:])
                    for tb in range(4):
                        pv = pp1.tile([128, 256], F32, name="pv", tag="pv")
                        for kk in range(KT):
                            nc.tensor.matmul(
                                pv[:],
                                xb[:, kk, ds(512 * n + 128 * tb, 128)],
                                wv_sb[:, kk, :],
                                start=kk == 0,
                                stop=kk == KT - 1,
                            )
                        drain(v_sb[:, 4 * n + tb, :], pv[:])

            # ---- phases 2+3: attention + output projection ----
            with (
                tc.tile_pool(name="ps_s", bufs=2, space="PSUM") as pps,
                tc.tile_pool(name="ps_o", bufs=2, space="PSUM") as ppo,
                tc.tile_pool(name="ps_db", bufs=2, space="PSUM") as ppdb,
                tc.tile_pool(name="ps_y", bufs=2, space="PSUM") as ppy,
            ):
                at_tiles = {}
                tails = {}

                def emit_heads(
                    j, hh, q_sb=q_sb, k_sb=k_sb, v_sb=v_sb, ql=ql_sb, kl=kl_sb
                ):
                    nblk = 4 * j + 4
                    po = ppo.tile([128, 512], F32, name="po", tag="po")
                    pd = ppdb.tile([1, 512], F32, name="pd", tag="pdb")
                    psbs = [None] * nblk

                    def s_mm(i):
                        psb = pps.tile([128, 512], F32, name="psb", tag="ps")
                        kh = k_sb[:, hh, ds(128 * i, 128)]
                        qh = q_sb[:, hh, ts(j, 512)]
                        if HILO_J0 and j == 0 and i == 0:
                            # hi+lo compensated scores for the t<128 rows
                            nc.tensor.matmul(psb[:], kh, qh, start=True, stop=False)
                            nc.tensor.matmul(
                                psb[:],
                                kl[:, hh, ds(0, 128)],
                                qh,
                                start=False,
                                stop=False,
                            )
                            nc.tensor.matmul(
                                psb[:], kh, ql[:, hh, :], start=False, stop=True
                            )
                        else:
                            nc.tensor.matmul(psb[:], kh, qh, start=True, stop=True)
                        psbs[i] = psb

                    s_mm(0)
                    wprev = [None]
                    for i in range(nblk):
                        if i + 1 < nblk:
                            s_mm(i + 1)
                        w_t = wp.tile([128, 512], BF, name="w_t", tag="w")
                        relu_drain(w_t, psbs[i])
                        r = i - 4 * j
                        if r >= 0:  # diagonal block: causal 0/1 mask
                            nc.vector.tensor_mul(w_t[:], w_t[:], mask_sb[:, r, :])
                        nc.tensor.matmul(
                            po[:],
                            v_sb[:, i, ts(hh, 128)],
                            w_t[:],
                            start=i == 0,
                            stop=i == nblk - 1,
                        )
                        nc.tensor.matmul(
                            pd[:],
                            ones_col[:],
                            w_t[:],
                            start=i == 0,
                            stop=i == nblk - 1,
                        )
                    # tail part A (no PE): free the po/pd banks right away and
                    # run the reciprocal off the critical path
                    po_sb = posp.tile([128, 512], F32, name="po_sb", tag="po_sb")
                    nc.scalar.copy(po_sb[:], po[:])
                    den = smallp.tile([1, 512], F32, name="den", tag="den")
                    nc.vector.tensor_scalar_add(den[:], pd[:], EPS)
                    rec_r = recp.tile([1, 512], BF, name="rec_r", tag="rec_r")
                    with nc.allow_low_precision(
                        reason="bf16 reciprocal feeds bf16 matmul broadcast"
                    ):
                        nc.vector.reciprocal(rec_r[:], den[:])
                    tails[(j, hh)] = (po_sb, rec_r)

                def finish_tail(j, hh):
                    # part B: the rank-1 broadcast matmul sits deep in the PE
                    # queue by now, so its input chain is long since resolved
                    po_sb, rec_r = tails.pop((j, hh))
                    pbc = ppdb.tile([128, 512], F32, name="pbc", tag="pdb")
                    nc.tensor.matmul(
                        pbc[:], ones_row_b[:], rec_r[:], start=True, stop=True
                    )
                    bc_sb = wp.tile([128, 512], F32, name="bc_sb", tag="bc")
                    nc.scalar.copy(bc_sb[:], pbc[:])
                    at = attnp.tile([128, 512], BF, name=f"at{hh}", tag=f"attn{hh}")
                    nc.vector.tensor_mul(at[:], po_sb[:], bc_sb[:])
                    at_tiles[(j, hh)] = at

                def emit_outproj(j, b=b):
                    a0 = at_tiles.pop((j, 0))
                    a1 = at_tiles.pop((j, 1))
                    for s in range(4):
                        yst = ystp.tile([128, C], F32, name="yst", tag="yst")
                        for ot in range(4):
                            py = ppy.tile([128, 512], F32, name="py", tag="py")
                            nc.tensor.matmul(
                                py[:],
                                a0[:, ts(s, 128)],
                                wo_sb[:, 0, ts(ot, 512)],
                                start=True,
                                stop=False,
                            )
                            nc.tensor.matmul(
                                py[:],
                                a1[:, ts(s, 128)],
                                wo_sb[:, 1, ts(ot, 512)],
                                start=False,
                                stop=True,
                            )
                            drain(yst[:, ts(ot, 512)], py[:])
                        nc.sync.dma_start(
                            y.ap()[ds(T * b + 512 * j + 128 * s, 128), :], yst[:]
                        )

                # finish_tail(j) runs behind blocks(j+1): by then the slow
                # [1,512] reciprocal chain feeding its broadcast has resolved
                for j in range(NT):
                    emit_heads(j, 0)
                    emit_heads(j, 1)
                    if j > 0:
                        finish_tail(j - 1, 0)
                        finish_tail(j - 1, 1)
                        emit_outproj(j - 1)
                finish_tail(NT - 1, 0)
                finish_tail(NT - 1, 1)
                emit_outproj(NT - 1)
    if split_waits:
        split_excess_waits(nc)
    return nc


def _host_masks():
    p = np.arange(128, dtype=np.int32)[:, None]
    f = np.arange(512, dtype=np.int32)[None, :]
    return np.stack(
        [(f >= 128 * r + p).astype(np.float32) for r in range(4)], axis=0
    )


def kernel(x, Wq, Wk, Wv, Wo, _trace=False, _trace_kwargs=None):
    global _NC_CACHE
    import ml_dtypes

    bf16 = ml_dtypes.bfloat16
    x = np.asarray(x, dtype=np.float32)
    Wq = np.asarray(Wq, dtype=np.float32) * SCALE  # fold attention scale
    Wk = np.asarray(Wk, dtype=np.float32)
    Wv = np.asarray(Wv, dtype=np.float32)
    Wo = np.asarray(Wo, dtype=np.float32)

    if _NC_CACHE is None:
        _NC_CACHE = _build()
    nc = _NC_CACHE

    xT = np.ascontiguousarray(x.reshape(B * T, C).T).astype(bf16)
    masks = _host_masks().astype(bf16)
    in_maps = []
    for c in range(N_CORES):
        sl = slice(CH * c, CH * (c + 1))
        in_maps.append(
            {
                "xT": xT,
                "wq": np.ascontiguousarray(Wq[sl, :].T).astype(bf16),
                "wk": np.ascontiguousarray(Wk[sl, :].T).astype(bf16),
                "wv": np.ascontiguousarray(Wv[sl, :].T).astype(bf16),
                "wo": np.ascontiguousarray(Wo[:, sl].T).astype(bf16),
                "masks": masks,
            }
        )

    res = run_bass_kernel_spmd(
        nc,
        in_maps,
        core_ids=list(range(N_CORES)),
        trace=_trace,
        **(_trace_kwargs or {}),
    )
    acc = np.zeros((B * T, C), dtype=np.float64)
    for c in range(N_CORES):
        acc += res.results[c]["y"].astype(np.float64)
    out = acc.astype(np.float32).reshape(B, T, C)
    if _trace:
        return out, res
    return out


# revision 37
# speedup vs baseline: 1.0250x; 1.0250x over previous
"""GhostAttention (B=2, T=2048, C=2048, H=16) on 8 Trainium2 NeuronCores.

Sharding: tensor-parallel over heads (Megatron-style). Core c owns heads
{2c, 2c+1}: it gets the 256 matching rows of Wq/Wk/Wv (column-parallel) and
the 256 matching columns of Wo (row-parallel), computes QKV projections,
masked-relu attention and its partial output projection for both batches,
and writes a full-shape partial y. The host sums the 8 partials.

v2 (bf16 + PE-continuity schedule):
  All matmul operands are bf16 (same PE rate as fp32r, half the SBUF/DMA
  traffic; enables fast DVE ops on 16-bit tiles). The attention scale is
  folded into Wq on the host.
  phase 1: the batch's full x^T lives in SBUF (64KB/partition in bf16), so
           each projection quantity (q/k per head, v per 128-token block)
           accumulates as its own full-bank PSUM group through a 2-bank
           ring -- PSUM allows only one accumulation group per 2KB bank.
           No drain bubbles; drains alternate ACT/DVE; weights arrive in
           4 k-groups so the first matmul starts ~2us in.
  phase 2: S^T blocks (tk=128, tq=512) with the S matmul emitted one block
           ahead of the relu+AV pair; relu (bias folded) alternates between
           ACT and DVE so drain throughput ~2x the PE block rate; diagonal
           blocks get a 0/1 mask multiply on DVE (bf16, 4x mode). AV and a
           ones-column normalizer matmul accumulate per block; the
           reciprocal is broadcast with a rank-1 matmul and applied on DVE
           directly PSUM*PSUM -> bf16 attn tile.
  phase 3: out-projection interleaved between the two head-groups of the
           next j-tile to keep the PE queue deep; PSUM drains alternate
           ACT/DVE; y staged in f32 and DMA'd per 128-token row block.
"""

import math
import sys

if "/opt/trn_rl_repo" not in sys.path:
    sys.path.insert(0, "/opt/trn_rl_repo")

import numpy as np
from contextlib import ExitStack

import concourse.bass as bass
import concourse.mybir as mybir
import concourse.tile as tile
from concourse.bass import ts, ds
from concourse.bass_utils import run_bass_kernel_spmd
from concourse.vector_clock import ScopedClock, VectorClock


def _split_drain_and_barrier(self, tick_clock, wait_clock):
    # This image's walrus caps sem waits per instruction; split the Tile-tail
    # drain waits across single-wait SP nops instead.
    gc = tick_clock.global_clock
    n = len(gc)
    for proc in range(n):
        t = gc[proc]
        if t <= 0:
            continue
        vc = VectorClock([0] * n)
        vc.require_at_least(proc, t)
        nop_inst = self.nc.sync.nop()
        wait_clock.add_sem_waits(nop_inst.ins, ScopedClock({None: vc}))
    self.nc.sync.drain()
    self.nc.all_engine_barrier()
    assert self.sems is not None
    popped = self.nc._tile_sem_poison_stack.pop()
    assert popped is self._sem_poison
    self.nc.clear_and_free_semaphores(list(self.sems.allocated().values()))
    self.nc.all_engine_barrier()


tile.TileContext._drain_and_barrier = _split_drain_and_barrier

_ws_counter = [0]


def split_excess_waits(nc, max_waits=1):
    """Hoist extra per-instruction sem waits onto preceding same-engine NoOps
    (same queue => they execute, and therefore wait, before the instruction)."""
    for fn in nc.m.functions:
        for blk in fn.blocks:
            insts = list(blk.instructions)
            out = []
            changed = False
            for inst in insts:
                si = inst.sync_info
                if si is not None and si.on_wait and len(si.on_wait) > max_waits:
                    waits = list(si.on_wait)
                    extra, keep = waits[:-max_waits], waits[-max_waits:]
                    for s in range(0, len(extra), max_waits):
                        chunk = extra[s : s + max_waits]
                        _ws_counter[0] += 1
                        nop = mybir.InstNoOp(
                            name=f"I-ws-{_ws_counter[0]}",
                            engine=inst.engine,
                            ins=[],
                            outs=[],
                            sync_info=mybir.SyncInfo(on_wait=chunk, on_update=[]),
                        )
                        out.append(nop)
                    inst.sync_info = mybir.SyncInfo(
                        on_wait=keep, on_update=list(si.on_update)
                    )
                    changed = True
                out.append(inst)
            if changed:
                try:
                    blk.instructions[:] = out
                except Exception:
                    blk.set_instructions(out)
    return nc


B, T, C = 2, 2048, 2048
H = 16
HD = C // H  # 128
N_CORES = 8
H_PER_CORE = H // N_CORES  # 2
CH = HD * H_PER_CORE  # 256 channels per core
SCALE = 1.0 / math.sqrt(HD)
ATTN_BIAS = 0.1  # relu(scores - (-0.1)) = relu(scores + 0.1)
EPS = 1e-6

F32 = mybir.dt.float32
F32R = mybir.dt.float32r
BF = mybir.dt.bfloat16
AF = mybir.ActivationFunctionType
ALU = mybir.AluOpType

_NC_CACHE = None

KT = C // 128  # 16 contraction slices
NCH = T // 256  # 8 phase-1 chunks per batch
NT = T // 512  # 4 query tiles of 512 per batch
USE_F32R_J0 = False  # f32r j=0 scores: numerically right in sim, wrong on HW
HILO_J0 = False  # bisect flag: hi+lo compensated (j0,i0) S block


def _build(split_waits=True):
    nc = bass.Bass("TRN2", debug=False)
    xT = nc.dram_tensor("xT", [C, B * T], BF, kind="ExternalInput")
    wq = nc.dram_tensor("wq", [C, CH], BF, kind="ExternalInput")  # pre-scaled
    wk = nc.dram_tensor("wk", [C, CH], BF, kind="ExternalInput")
    wv = nc.dram_tensor("wv", [C, CH], BF, kind="ExternalInput")
    wo = nc.dram_tensor("wo", [CH, C], BF, kind="ExternalInput")
    masks = nc.dram_tensor("masks", [4, 128, 512], BF, kind="ExternalInput")
    y = nc.dram_tensor("y", [B * T, C], F32, kind="ExternalOutput")

    with tile.TileContext(nc) as tc, ExitStack() as ctx:
        consts = ctx.enter_context(tc.tile_pool(name="consts", bufs=1))
        qkvp = ctx.enter_context(tc.tile_pool(name="qkv", bufs=1))
        xinp = ctx.enter_context(tc.tile_pool(name="xin", bufs=1))
        wp = ctx.enter_context(tc.tile_pool(name="wtile", bufs=4))
        attnp = ctx.enter_context(tc.tile_pool(name="attn", bufs=2))
        ystp = ctx.enter_context(tc.tile_pool(name="yst", bufs=2))
        smallp = ctx.enter_context(tc.tile_pool(name="small", bufs=2))
        recp = ctx.enter_context(tc.tile_pool(name="recp", bufs=4))
        posp = ctx.enter_context(tc.tile_pool(name="posb", bufs=4))
        resp = ctx.enter_context(tc.tile_pool(name="resid", bufs=2))

        wq_sb = consts.tile([128, KT, CH], BF, name="wq_sb", tag="wq")
        wk_sb = consts.tile([128, KT, CH], BF, name="wk_sb", tag="wk")
        wv_sb = consts.tile([128, KT, CH], BF, name="wv_sb", tag="wv")
        # 4 k-slice groups per weight so the first matmuls start early.
        for g in range(4):
            rs = ds(512 * g, 512)
            gs = ds(4 * g, 4)
            for w_d, w_s in ((wq_sb, wq), (wk_sb, wk), (wv_sb, wv)):
                nc.sync.dma_start(
                    w_d[:, gs, :],
                    w_s.ap()[rs, :].rearrange("(k p) o -> p k o", p=128),
                )
        wo_sb = consts.tile([128, H_PER_CORE, C], BF, name="wo_sb", tag="wo")
        nc.sync.dma_start(wo_sb[:], wo.ap().rearrange("(h p) o -> p h o", p=128))
        mask_sb = consts.tile([128, 4, 512], BF, name="mask_sb", tag="masks")
        for r in range(4):
            nc.sync.dma_start(mask_sb[:, r, :], masks.ap()[r])
        ones_col = consts.tile([128, 1], BF, name="ones_col", tag="ones_col")
        nc.vector.memset(ones_col[:], 1.0)
        ones_row_b = consts.tile([1, 128], BF, name="ones_row_b", tag="ones_row_b")
        nc.vector.memset(ones_row_b[:], 1.0)
        bias_sb = consts.tile([128, 1], F32, name="bias_sb", tag="bias")
        nc.vector.memset(bias_sb[:], ATTN_BIAS)

        xT_re = xT.ap().rearrange("(k p) t -> p k t", p=128)  # (128, KT, B*T)

        # global ACT/DVE alternation for PSUM drains
        par = [0]

        def drain(dst, src):
            # 3:2 ACT:DVE -- DVE also carries the reciprocal + mask work
            if par[0] % 5 < 3:
                nc.scalar.copy(dst, src)
            else:
                nc.vector.tensor_scalar_add(dst, src, 0.0)
            par[0] += 1

        relu_ctr = [0]

        def relu_drain(w_t, psb):
            # 3:2 ACT:DVE split -- ACT has more slack than DVE
            if relu_ctr[0] % 5 < 3:
                nc.scalar.activation(
                    w_t[:], psb[:], AF.Relu, bias=bias_sb[:], scale=1.0
                )
            else:
                nc.vector.tensor_scalar(
                    w_t[:], psb[:], ATTN_BIAS, 0.0, ALU.add, ALU.max
                )
            relu_ctr[0] += 1

        for b in range(B):
            q_sb = qkvp.tile([128, H_PER_CORE, T], BF, name="q_sb", tag="q")
            k_sb = qkvp.tile([128, H_PER_CORE, T], BF, name="k_sb", tag="k")
            v_sb = qkvp.tile([128, T // 128, CH], BF, name="v_sb", tag="v")
            # bf16 residuals of q/k for the first 512 tokens: rows t<128 have
            # a tiny normalizer, so the S block (j=0,i=0) runs hi+lo
            # compensated (3 bf16 matmuls) to get near-f32 scores there.
            # Own pool with bufs=2: no cross-batch buffer reuse.
            ql_sb = resp.tile([128, H_PER_CORE, 512], BF, name="ql", tag="ql")
            kl_sb = resp.tile([128, H_PER_CORE, 512], BF, name="kl", tag="kl")

            # ---- phase 1: x resident in SBUF; one PSUM group per bank ----
            xb = xinp.tile([128, KT, T], BF, name="xb", tag="xb")
            for n in range(NT):
                for kk in range(KT):
                    nc.sync.dma_start(
                        xb[:, kk, ts(n, 512)],
                        xT_re[:, kk, ds(T * b + 512 * n, 512)],
                    )
            with tc.tile_pool(name="ps1", bufs=2, space="PSUM") as pp1:
                for n in range(NT):
                    for w_sb, dst, dst32 in (
                        (wk_sb, k_sb, kl_sb),
                        (wq_sb, q_sb, ql_sb),
                    ):
                        for h in (0, 1):
                            pqk = pp1.tile([128, 512], F32, name="pqk", tag="pqk")
                            for kk in range(KT):
                                nc.tensor.matmul(
                                    pqk[:],
                                    w_sb[:, kk, ts(h, 128)],
                                    xb[:, kk, ds(512 * n, 512)],
                                    start=kk == 0,
                                    stop=kk == KT - 1,
                                )
                            if n == 0:
                                # single PSUM read to f32 SBUF, then derive
                                # bf16 hi + bf16 residual lo from SBUF
                                qf = resp.tile(
                                    [128, 512], F32, name="qf32", tag="qf32"
                                )
                                nc.scalar.copy(qf[:], pqk[:])
                                nc.scalar.copy(dst[:, h, ts(0, 512)], qf[:])
                                nc.vector.scalar_tensor_tensor(
                                    dst32[:, h, :],
                                    qf[:],
                                    1.0,
                                    dst[:, h, ts(0, 512)],
                                    op0=ALU.mult,
                                    op1=ALU.subtract,
                                )
                            else:
                                drain(dst[:, h, ts(n, 512)], pqk[:])
                    for tb in range(4):
                        pv = pp1.tile([128, 256], F32, name="pv", tag="pv")
                        for kk in range(KT):
                            nc.tensor.matmul(
                                pv[:],
                                xb[:, kk, ds(512 * n + 128 * tb, 128)],
                                wv_sb[:, kk, :],
                                start=kk == 0,
                                stop=kk == KT - 1,
                            )
                        drain(v_sb[:, 4 * n + tb, :], pv[:])

            # ---- phases 2+3: attention + output projection ----
            with (
                tc.tile_pool(name="ps_s", bufs=2, space="PSUM") as pps,
                tc.tile_pool(name="ps_o", bufs=2, space="PSUM") as ppo,
                tc.tile_pool(name="ps_db", bufs=2, space="PSUM") as ppdb,
                tc.tile_pool(name="ps_y", bufs=2, space="PSUM") as ppy,
            ):
                at_tiles = {}
                tails = {}

                def emit_heads(
                    j, hh, q_sb=q_sb, k_sb=k_sb, v_sb=v_sb, ql=ql_sb, kl=kl_sb
                ):
                    nblk = 4 * j + 4
                    po = ppo.tile([128, 512], F32, name="po", tag="po")
                    pd = ppdb.tile([1, 512], F32, name="pd", tag="pdb")
                    psbs = [None] * nblk

                    def s_mm(i):
                        psb = pps.tile([128, 512], F32, name="psb", tag="ps")
                        kh = k_sb[:, hh, ds(128 * i, 128)]
                        qh = q_sb[:, hh, ts(j, 512)]
                        if HILO_J0 and j == 0 and i == 0:
                            # hi+lo compensated scores for the t<128 rows
                            nc.tensor.matmul(psb[:], kh, qh, start=True, stop=False)
                            nc.tensor.matmul(
                                psb[:],
                                kl[:, hh, ds(0, 128)],
                                qh,
                                start=False,
                                stop=False,
                            )
                            nc.tensor.matmul(
                                psb[:], kh, ql[:, hh, :], start=False, stop=True
                            )
                        else:
                            nc.tensor.matmul(psb[:], kh, qh, start=True, stop=True)
                        psbs[i] = psb

                    s_mm(0)
                    wprev = [None]
                    for i in range(nblk):
                        if i + 1 < nblk:
                            s_mm(i + 1)
                        w_t = wp.tile([128, 512], BF, name="w_t", tag="w")
                        relu_drain(w_t, psbs[i])
                        r = i - 4 * j
                        if r >= 0:  # diagonal block: causal 0/1 mask
                            nc.vector.tensor_mul(w_t[:], w_t[:], mask_sb[:, r, :])
                        nc.tensor.matmul(
                            po[:],
                            v_sb[:, i, ts(hh, 128)],
                            w_t[:],
                            start=i == 0,
                            stop=i == nblk - 1,
                        )
                        if i % 2 == 0:
                            wprev[0] = w_t
                        else:
                            # pair-sum on DVE (bf16) halves the ones-matmul
                            # normalizer count on the PE
                            wsum = wp.tile([128, 512], BF, name="wsum", tag="wsum")
                            nc.vector.tensor_add(wsum[:], wprev[0][:], w_t[:])
                            nc.tensor.matmul(
                                pd[:],
                                ones_col[:],
                                wsum[:],
                                start=i == 1,
                                stop=i == nblk - 1,
                            )
                    # tail part A (no PE): free the po/pd banks right away and
                    # run the reciprocal off the critical path
                    po_sb = posp.tile([128, 512], F32, name="po_sb", tag="po_sb")
                    nc.scalar.copy(po_sb[:], po[:])
                    den = smallp.tile([1, 512], F32, name="den", tag="den")
                    nc.vector.tensor_scalar_add(den[:], pd[:], EPS)
                    rec_r = recp.tile([1, 512], BF, name="rec_r", tag="rec_r")
                    with nc.allow_low_precision(
                        reason="bf16 reciprocal feeds bf16 matmul broadcast"
                    ):
                        nc.vector.reciprocal(rec_r[:], den[:])
                    tails[(j, hh)] = (po_sb, rec_r)

                def finish_tail(j, hh):
                    # part B: the rank-1 broadcast matmul sits deep in the PE
                    # queue by now, so its input chain is long since resolved
                    po_sb, rec_r = tails.pop((j, hh))
                    pbc = ppdb.tile([128, 512], F32, name="pbc", tag="pdb")
                    nc.tensor.matmul(
                        pbc[:], ones_row_b[:], rec_r[:], start=True, stop=True
                    )
                    bc_sb = wp.tile([128, 512], F32, name="bc_sb", tag="bc")
                    nc.scalar.copy(bc_sb[:], pbc[:])
                    at = attnp.tile([128, 512], BF, name=f"at{hh}", tag=f"attn{hh}")
                    nc.vector.tensor_mul(at[:], po_sb[:], bc_sb[:])
                    at_tiles[(j, hh)] = at

                def emit_outproj(j, b=b):
                    a0 = at_tiles.pop((j, 0))
                    a1 = at_tiles.pop((j, 1))
                    for s in range(4):
                        yst = ystp.tile([128, C], F32, name="yst", tag="yst")
                        for ot in range(4):
                            py = ppy.tile([128, 512], F32, name="py", tag="py")
                            nc.tensor.matmul(
                                py[:],
                                a0[:, ts(s, 128)],
                                wo_sb[:, 0, ts(ot, 512)],
                                start=True,
                                stop=False,
                            )
                            nc.tensor.matmul(
                                py[:],
                                a1[:, ts(s, 128)],
                                wo_sb[:, 1, ts(ot, 512)],
                                start=False,
                                stop=True,
                            )
                            drain(yst[:, ts(ot, 512)], py[:])
                        nc.sync.dma_start(
                            y.ap()[ds(T * b + 512 * j + 128 * s, 128), :], yst[:]
                        )

                # finish_tail(j) runs behind blocks(j+1): by then the slow
                # [1,512] reciprocal chain feeding its broadcast has resolved
                for j in range(NT):
                    emit_heads(j, 0)
                    emit_heads(j, 1)
                    if j > 0:
                        finish_tail(j - 1, 0)
                        finish_tail(j - 1, 1)
                        emit_outproj(j - 1)
                finish_tail(NT - 1, 0)
                finish_tail(NT - 1, 1)
                emit_outproj(NT - 1)
    if split_waits:
        split_excess_waits(nc)
    return nc


def _host_masks():
    p = np.arange(128, dtype=np.int32)[:, None]
    f = np.arange(512, dtype=np.int32)[None, :]
    return np.stack(
        [(f >= 128 * r + p).astype(np.float32) for r in range(4)], axis=0
    )


def kernel(x, Wq, Wk, Wv, Wo, _trace=False, _trace_kwargs=None):
    global _NC_CACHE
    import ml_dtypes

    bf16 = ml_dtypes.bfloat16
    x = np.asarray(x, dtype=np.float32)
    Wq = np.asarray(Wq, dtype=np.float32) * SCALE  # fold attention scale
    Wk = np.asarray(Wk, dtype=np.float32)
    Wv = np.asarray(Wv, dtype=np.float32)
    Wo = np.asarray(Wo, dtype=np.float32)

    if _NC_CACHE is None:
        _NC_CACHE = _build()
    nc = _NC_CACHE

    xT = np.ascontiguousarray(x.reshape(B * T, C).T).astype(bf16)
    masks = _host_masks().astype(bf16)
    in_maps = []
    for c in range(N_CORES):
        sl = slice(CH * c, CH * (c + 1))
        in_maps.append(
            {
                "xT": xT,
                "wq": np.ascontiguousarray(Wq[sl, :].T).astype(bf16),
                "wk": np.ascontiguousarray(Wk[sl, :].T).astype(bf16),
                "wv": np.ascontiguousarray(Wv[sl, :].T).astype(bf16),
                "wo": np.ascontiguousarray(Wo[:, sl].T).astype(bf16),
                "masks": masks,
            }
        )

    res = run_bass_kernel_spmd(
        nc,
        in_maps,
        core_ids=list(range(N_CORES)),
        trace=_trace,
        **(_trace_kwargs or {}),
    )
    acc = np.zeros((B * T, C), dtype=np.float64)
    for c in range(N_CORES):
        acc += res.results[c]["y"].astype(np.float64)
    out = acc.astype(np.float32).reshape(B, T, C)
    if _trace:
        return out, res
    return out
